# revision 4
# baseline (speedup 1.0000x reference)
"""Trainium2 Bass kernel for 4D convolution (3x3x3x3, pad 1, stride 1).

Problem: x (2, 8, 7, 7, 48, 48) f32, conv (8, 648) f32, bias (8,) f32
         -> out (2, 8, 7, 7, 48, 48) f32.

Sharding: 8 cores = (batch b in {0,1}) x (h-chunk hc in {0..3}, 12 rows).

Per core: 16-way 32x32 tile_position packing of the PE array.

The (s -> t) banded weight structure (output h-row t contracts input
h-rows s in {t-1, t, t+1}, 8 channels each) is covered exactly by four
32x32 rectangles, each 9 dense 8x8 blocks:

  R0: T={0,1,2,3}   S={-1,0,1,2}   cols [t2,t3,t0,t1]
  R1: T={2,3,4,5}   S={3,4,5,6}    cols [t2,t3,t4,t5]
  R2: T={6,7,8,9}   S={5,6,7,8}    cols [t8,t9,t6,t7]
  R3: T={8,9,10,11} S={9,10,11,12} cols [t8,t9,t10,t11]

Rectangle r reads x rows stored at SBUF partitions 32r..32r+31
(s-window-major, s rows 5,6 duplicated across groups 1/2) and runs as a
PE tile at tile_position (32r, 32*cg), cg = (r + k(u)) % 4.  Four lanes
k in 0..3 each own u's {0,4}, {1,5}, {2,6}, {3}: at any moment 16
tiles stream concurrently (4 rectangles x 4 lanes), each a K=32, M=32,
N=336 matmul -> ~4x PE throughput vs the previous 2-stream kernel.

t2,t3 are split across R0/R1 and t8,t9 across R2/R3; the drain is one
128-row ACT (bias, psum->bf16) plus two 16-row DVE merge adds reading
raw psum, then a full 128-row DMA out; the host gathers the 96 real
rows per (core, u).
"""

import sys

if "/opt/trn_rl_repo" not in sys.path:
    sys.path.insert(0, "/opt/trn_rl_repo")

import numpy as np
import ml_dtypes

B, C, OC = 2, 8, 8
U, V, H, W = 7, 7, 48, 48
TH = 12
NCHUNKS = H // TH
NCORES = B * NCHUNKS
NCOL = V * W        # 336
XROW = (V + 2) * (W + 2)  # 450
XFREE = U * XROW

# rectangle r: (T list in column order, S list in partition order)
RECTS = [
    ([2, 3, 0, 1], [-1, 0, 1, 2]),
    ([2, 3, 4, 5], [3, 4, 5, 6]),
    ([8, 9, 6, 7], [5, 6, 7, 8]),
    ([8, 9, 10, 11], [9, 10, 11, 12]),
]

SH_ORDER = [
    (i0, i1, i3) for i0 in (1, 2, 0) for i1 in range(3) for i3 in range(3)
]

# lane k owns these u values (processed in order)
LANE_US = [[0, 4], [1, 5], [2, 6], [3]]

N_WARMUP_MM = 4

_built = {}


def _build_nc(reps=None):
    import contextlib

    import concourse.bacc as bacc
    import concourse.mybir as mybir
    from concourse.tile import TileContext

    BF16 = mybir.dt.bfloat16
    F32 = mybir.dt.float32

    nc = bacc.Bacc(
        "TRN2", target_bir_lowering=False, debug=False, num_devices=NCORES
    )
    xw_d = nc.dram_tensor("xw", [128, XFREE], BF16, kind="ExternalInput")
    wt_d = nc.dram_tensor("wt", [128, 27 * 32], BF16, kind="ExternalInput")
    bias_d = nc.dram_tensor("bias", [128, 1], F32, kind="ExternalInput")
    out_d = nc.dram_tensor("out", [128, U * NCOL], BF16, kind="ExternalOutput")

    with TileContext(nc) as tc:
        with (
            tc.tile_pool(name="sbuf", bufs=1) as pool,
            tc.tile_pool(name="psum", bufs=1, space="PSUM") as pp,
        ):
            loop = tc.For_i(0, reps, 1) if reps is not None else contextlib.nullcontext()
            with loop:
                scr = pool.tile([128, 512], BF16, tag="scr")
                nc.gpsimd.memset(scr[:], 0.0)
                ps_warm = pp.tile([128, 1024], F32, tag="ps3", bufs=1,
                                  name="ps_lane3")
                for _ in range(N_WARMUP_MM):
                    nc.tensor.matmul(
                        ps_warm[0:32, 512:848], scr[0:32, :32],
                        scr[0:32, :336], start=True, stop=True,
                        tile_position=(0, 0),
                    )

                w_first = pool.tile([128, 32], BF16, tag="wf", name="w_first")
                w_sb = pool.tile([128, 26 * 32], BF16, tag="w", name="w_sb")
                x_sb = pool.tile([128, XFREE], BF16, tag="x", name="x_sb")
                b_sb = pool.tile([128, 1], F32, tag="b")
                nc.scalar.dma_start(out=w_first[:], in_=wt_d[:, 0:32])
                nc.sync.dma_start(
                    out=x_sb[:, 0 : 5 * XROW], in_=xw_d[:, 0 : 5 * XROW]
                )
                nc.scalar.dma_start(out=w_sb[:], in_=wt_d[:, 32:])
                nc.scalar.dma_start(
                    out=x_sb[:, 5 * XROW :], in_=xw_d[:, 5 * XROW :]
                )
                nc.scalar.dma_start(out=b_sb[:], in_=bias_d[:])

                # per-lane psum pair tiles: lane's u_a at cols 0-335,
                # u_b at cols 512-847 (warmup tile doubles as lane 3's)
                lane_ps = [
                    pp.tile([128, 1024], F32, tag=f"ps{k}", bufs=1,
                            name=f"ps_lane{k}")
                    for k in range(3)
                ] + [ps_warm]

                def lhsT_for(pos, r):
                    rows = slice(32 * r, 32 * r + 32)
                    if pos == 0:
                        return w_first[rows, :]
                    return w_sb[rows, (pos - 1) * 32 : pos * 32]

                def rhs_for(u, i0, i1, i3, r):
                    return (
                        x_sb[
                            32 * r : 32 * r + 32,
                            (u + i0 - 1) * XROW : (u + i0) * XROW,
                        ]
                        .rearrange("p (v w) -> p v w", v=V + 2)
                        [:, i1 : i1 + V, i3 : i3 + W]
                    )

                o_sb = pool.tile([128, U * NCOL], BF16, tag="o", name="o_sb")

                # build per-lane task lists: (u, slot, pos, i0, i1, i3)
                lane_tasks = []
                for k in range(4):
                    tasks = []
                    for slot, u in enumerate(LANE_US[k]):
                        shifts = [
                            (pos, i0, i1, i3)
                            for pos, (i0, i1, i3) in enumerate(SH_ORDER)
                            if 1 <= u + i0 <= 7
                        ]
                        for idx, (pos, i0, i1, i3) in enumerate(shifts):
                            tasks.append((
                                u, slot, pos, i0, i1, i3,
                                idx == 0, idx == len(shifts) - 1,
                            ))
                    lane_tasks.append(tasks)

                def drain(k, u, slot):
                    off = 512 * slot
                    ucols = slice(u * NCOL, (u + 1) * NCOL)
                    pst = lane_ps[k]
                    # single strided ACT: bias + psum -> bf16, all 128 rows
                    nc.scalar.activation(
                        out=o_sb[:, ucols],
                        in_=pst[:, off : off + NCOL],
                        func=mybir.ActivationFunctionType.Identity,
                        bias=b_sb[:, :],
                    )
                    # merge split t-pairs: dest rows = R1/R3 block rows
                    # 0-15 (already ACT'd with bias); in0 = raw psum of
                    # R0/R2 block rows 0-15 (never biased)
                    for pair in range(2):
                        gsrc = 32 * ((2 * pair + k) % 4)
                        gdst = 32 * ((2 * pair + 1 + k) % 4)
                        nc.vector.scalar_tensor_tensor(
                            out=o_sb[gdst : gdst + 16, ucols],
                            in0=pst[gsrc : gsrc + 16, off : off + NCOL],
                            scalar=0.0,
                            in1=o_sb[gdst : gdst + 16, ucols],
                            op0=mybir.AluOpType.add,
                            op1=mybir.AluOpType.add,
                        )
                    ring = nc.sync if (u % 2 == 0) else nc.scalar
                    ring.dma_start(
                        out=out_d[:, ucols], in_=o_sb[:, ucols]
                    )

                nsteps = max(len(t) for t in lane_tasks)
                for step in range(nsteps):
                    for r in range(4):
                        for k in range(4):
                            if step >= len(lane_tasks[k]):
                                continue
                            u, slot, pos, i0, i1, i3, first, last = \
                                lane_tasks[k][step]
                            cg = (r + k) % 4
                            nc.tensor.matmul(
                                lane_ps[k][
                                    32 * cg : 32 * cg + 32,
                                    512 * slot : 512 * slot + NCOL,
                                ],
                                lhsT_for(pos, r),
                                rhs_for(u, i0, i1, i3, r),
                                start=first,
                                stop=last,
                                tile_position=(32 * r, 32 * cg),
                            )
                    for k in range(4):
                        if step < len(lane_tasks[k]):
                            u, slot, pos, i0, i1, i3, first, last = \
                                lane_tasks[k][step]
                            if last:
                                drain(k, u, slot)

    nc.compile()
    return nc


def _get_nc():
    if "nc" not in _built:
        _built["nc"] = _build_nc()
    return _built["nc"]


def _build_weight_inputs(conv, bias):
    Wr = conv.reshape(OC, 3, 3, 3, 3, C).astype(np.float32)
    # wt[32r + 8*si + c, pos*32 + 8*tj + o] = Wr[o,i0,i1,i2,i3,c],
    # i2 = S_r[si] - T_r[tj] + 1 when in 0..2, else 0
    wt = np.zeros((128, 27, 32), np.float32)
    for r, (tlist, slist) in enumerate(RECTS):
        for si, s in enumerate(slist):
            for tj, t in enumerate(tlist):
                i2 = s - t + 1
                if 0 <= i2 <= 2:
                    for pos, (i0, i1, i3) in enumerate(SH_ORDER):
                        p0 = 32 * r + 8 * si
                        wt[p0 : p0 + 8, pos, 8 * tj : 8 * tj + 8] = Wr[
                            :, i0, i1, i2, i3, :
                        ].T
    wt = np.ascontiguousarray(
        wt.reshape(128, 27 * 32).astype(ml_dtypes.bfloat16)
    )
    # bias rows: bias[o] tiled over all 128 partitions (p%8 = o)
    bias_in = np.tile(
        bias.astype(np.float32), 16
    ).reshape(128, 1)
    return wt, bias_in


def _build_x_inputs(x):
    xh = np.zeros((B, C, U, V, H + 2, W), np.float32)
    xh[:, :, :, :, 1 : H + 1, :] = x
    xs = []
    for core in range(NCORES):
        b, hc = divmod(core, NCHUNKS)
        # slab rows s=-1..12 map to padded indices hc*TH + (s+1)
        slab = xh[b, :, :, :, hc * TH : hc * TH + TH + 2, :]  # (C,U,V,14,W)
        xc = np.zeros((C, TH + 2, U, V + 2, W + 2), np.float32)
        xc[:, :, :, 1 : V + 1, 1 : W + 1] = slab.transpose(0, 3, 1, 2, 4)
        sm = xc.transpose(1, 0, 2, 3, 4)  # (14, C, U, V+2, W+2)
        x128 = np.empty((128, XFREE), np.float32)
        for r, (_, slist) in enumerate(RECTS):
            for si, s in enumerate(slist):
                p0 = 32 * r + 8 * si
                x128[p0 : p0 + 8] = sm[s + 1].reshape(C, XFREE)
        xs.append(
            np.ascontiguousarray(x128.astype(ml_dtypes.bfloat16))
        )
    return xs


def _gather_rows(k):
    """o_sb row indices for t = 0..11 (each 8 rows) given lane offset k."""
    g = [32 * ((j + k) % 4) for j in range(4)]
    rows = []
    # R0 block: [t2,t3,t0,t1] -> t0 at +16, t1 at +24
    # R1 block: [t2,t3,t4,t5] -> merged t2 at +0, t3 at +8, t4 +16, t5 +24
    rows += [g[0] + 16, g[0] + 24]          # t0, t1
    rows += [g[1] + 0, g[1] + 8]            # t2, t3 (merged)
    rows += [g[1] + 16, g[1] + 24]          # t4, t5
    rows += [g[2] + 16, g[2] + 24]          # t6, t7
    rows += [g[3] + 0, g[3] + 8]            # t8, t9 (merged)
    rows += [g[3] + 16, g[3] + 24]          # t10, t11
    return rows


K_OF_U = {}
for _k, _us in enumerate(LANE_US):
    for _u in _us:
        K_OF_U[_u] = _k


def kernel(x, conv, bias):
    from concourse.bass_utils import run_bass_kernel_spmd

    nc = _get_nc()
    wt, bias_in = _build_weight_inputs(np.asarray(conv), np.asarray(bias))
    xs = _build_x_inputs(np.asarray(x, dtype=np.float32))
    in_maps = [{"xw": xc, "wt": wt, "bias": bias_in} for xc in xs]
    res = run_bass_kernel_spmd(nc, in_maps, core_ids=list(range(NCORES)))

    out = np.empty((B, OC, U, V, H, W), np.float32)
    for core in range(NCORES):
        b, hc = divmod(core, NCHUNKS)
        r = np.asarray(
            res.results[core]["out"], dtype=np.float32
        ).reshape(128, U, V, W)
        for u in range(U):
            rows = _gather_rows(K_OF_U[u])
            for t in range(TH):
                # rows[t] .. rows[t]+7 = the 8 output channels of t
                out[b, :, u, :, hc * TH + t, :] = r[
                    rows[t] : rows[t] + 8, u
                ]
    return out
